# revision 2
# baseline (speedup 1.0000x reference)
"""Trainium2 Bass kernel for CircleProjectionLayer (ball projection, r=1).

out = center + d * min(1, 1/||d||),  d = x - center,  shapes [8388608, 3] f32.

Sharding: pure data parallel — batch split 8 ways, one shard per NeuronCore.
Per-core layout: the [1048576, 3] shard is viewed flat as [128, 24576] so each
SBUF partition holds 8192 complete (x,y,z) rows contiguously; 16 chunks of
W=1536 floats per partition stream through SBUF.

The body is software-pipelined ("stage-skewed"): emission iteration t issues
stage S for chunk t - skew[S], so every engine's in-order queue only consumes
results produced >= 1 iteration earlier. This removes the head-of-line
blocking of naive per-chunk emission (engines ping-ponging DVE<->ACT within a
chunk), which measured ~55% above the pure-DMA floor (~113us/core for the
37.75 MB/core of HBM traffic at ~336 GB/s effective).

Stages per chunk i (W floats/partition, R = W/3 rows):
  A: dma x(i)->xt(i) [SP HWDGE ring], dma c(i)->ct(i) [ACT HWDGE ring]
  B: DVE  xt(i) = xt(i) - ct(i)                       (d, in place)
  C: ACT  sq(i) = Square(xt(i))
  D: DVE  ta(i) = sq0+sq1 ; tb(i) = max(ta,eps)+sq2   (scalar_tensor_tensor)
  E: ACT  Ln -> Relu -> Exp(-0.5*)  => sc(i)          (= min(1, rsqrt(ssq)),
     exact clamp at 1: relu(ln s)=0 for s<=1)
  F: DVE  sq(i) = d * broadcast(sc(i))                (one stride-0 mul over W)
  G: DVE/Pool  sq(i) += ct(i)                         (out; 50/50 split)
  H: SP-ring dma out(i) <- sq(i)   (HWDGE — keeps SWDGE descriptor
     generation off the Q7 cores, which also run the Pool half of G)

Engine busy/chunk ~= DVE 6.4us, ACT 4.4us, Pool 2.1us — measured full-kernel
148.4us vs 174.2us for the unskewed baseline and ~113us DMA-only floor.
"""

import sys

sys.path.insert(0, "/opt/trn_rl_repo")

from contextlib import ExitStack
import contextlib

import numpy as np

import concourse.bass as bass
import concourse.tile as tile
from concourse import bacc, mybir
from concourse.bass_utils import run_bass_kernel_spmd
from concourse.hw_specs import get_activation_tables

F32 = mybir.dt.float32
AF = mybir.ActivationFunctionType
ALU = mybir.AluOpType

B = 8388608
N_CORES = 8
B_CORE = B // N_CORES          # 1048576 rows per core
P = 128
FPP = B_CORE * 3 // P          # 24576 floats per partition

_EPS = 1e-30
_ACT_SET = "natural_log_exp_and_others"

_SCHEDULE = [1536] * 16


def _preload_act_table(nc):
    """Pre-place one LoadActFuncSet for the set containing Square/Ln/Relu/Exp
    so Bacc.insert_act_table_loads doesn't thrash between greedy choices."""
    tables = list(get_activation_tables(nc.m.arch).keys())
    set_id = tables.index(_ACT_SET)
    inst = mybir.InstLoadActFuncSet(
        name=nc.get_next_instruction_name(), act_func_set_id=set_id, ins=[], outs=[]
    )
    return nc.scalar.add_instruction(inst)


def _build(schedule=None, loop_reps=1,
           SB=2, SC=3, SD=4, SE=5, SF=6, SG=7, SH=8,
           bufs_xt=8, bufs_ct=9, bufs_sq=7, bufs_small=3,
           add_split=0.5, bcast_mul=True,
           x_ring="sp", c_ring="act", out_ring="sp"):
    """`loop_reps`: wrap the whole body in a hardware For_i loop (used only
    for benchmarking steady-state HW time via wall-clock deltas)."""
    if schedule is None:
        schedule = _SCHEDULE
    assert sum(schedule) == FPP and all(w % 3 == 0 for w in schedule)
    n = len(schedule)
    W = max(schedule)
    RW = W // 3

    nc = bacc.Bacc("TRN2", target_bir_lowering=False, debug=False)
    x = nc.dram_tensor("x", [B_CORE, 3], F32, kind="ExternalInput")
    c = nc.dram_tensor("center", [B_CORE, 3], F32, kind="ExternalInput")
    o = nc.dram_tensor("out", [B_CORE, 3], F32, kind="ExternalOutput")

    xr = x.ap().rearrange("(p f) c -> p (f c)", p=P)
    cr = c.ap().rearrange("(p f) c -> p (f c)", p=P)
    orr = o.ap().rearrange("(p f) c -> p (f c)", p=P)

    offs = [0]
    for w in schedule:
        offs.append(offs[-1] + w)

    with tile.TileContext(nc) as tc, ExitStack() as ctx:
        _preload_act_table(nc)
        xp = ctx.enter_context(tc.tile_pool(name="xp", bufs=bufs_xt))
        cp = ctx.enter_context(tc.tile_pool(name="cp", bufs=bufs_ct))
        sp = ctx.enter_context(tc.tile_pool(name="sp", bufs=bufs_sq))
        smp = ctx.enter_context(tc.tile_pool(name="smp", bufs=bufs_small))

        rings = {"sp": nc.sync, "act": nc.scalar, "pool": nc.gpsimd}

        loop_cm = tc.For_i(0, loop_reps, 1) if loop_reps > 1 else contextlib.nullcontext()
        with loop_cm:
            xt, ct, sq, ta, tb, sc = {}, {}, {}, {}, {}, {}
            for t in range(n + SH + 1):
                # A: input DMAs
                if t < n:
                    w, off = schedule[t], offs[t]
                    xt[t] = xp.tile([P, W], F32, name="xt", tag="xt")[:, :w]
                    rings[x_ring].dma_start(xt[t][:, :], xr[:, off : off + w])
                    ct[t] = cp.tile([P, W], F32, name="ct", tag="ct")[:, :w]
                    rings[c_ring].dma_start(ct[t][:, :], cr[:, off : off + w])
                # B: d = x - c (in place)
                i = t - SB
                if 0 <= i < n:
                    nc.vector.tensor_sub(xt[i][:, :], xt[i][:, :], ct[i][:, :])
                # C: squares
                i = t - SC
                if 0 <= i < n:
                    sq[i] = sp.tile([P, W], F32, name="sq", tag="sq")[:, : schedule[i]]
                    nc.scalar.activation(sq[i][:, :], xt[i][:, :], AF.Square)
                # D: row sums of squares
                i = t - SD
                if 0 <= i < n:
                    r = schedule[i] // 3
                    s3 = sq[i].rearrange("p (r c) -> p r c", c=3)
                    ta[i] = smp.tile([P, RW], F32, name="ta", tag="ta")[:, :r]
                    nc.vector.tensor_add(ta[i][:, :], s3[:, :, 0], s3[:, :, 1])
                    tb[i] = smp.tile([P, RW], F32, name="tb", tag="tb")[:, :r]
                    nc.vector.scalar_tensor_tensor(
                        tb[i][:, :], ta[i][:, :], _EPS, s3[:, :, 2], ALU.max, ALU.add
                    )
                # E: scale chain
                i = t - SE
                if 0 <= i < n:
                    r = schedule[i] // 3
                    nc.scalar.activation(ta[i][:, :], tb[i][:, :], AF.Ln)
                    nc.scalar.activation(tb[i][:, :], ta[i][:, :], AF.Relu)
                    sc[i] = smp.tile([P, RW], F32, name="sc", tag="sc")[:, :r]
                    nc.scalar.activation(sc[i][:, :], tb[i][:, :], AF.Exp, scale=-0.5)
                # F: m = d * s (one broadcast mul over the dead squares)
                i = t - SF
                if 0 <= i < n:
                    r = schedule[i] // 3
                    m3 = sq[i].rearrange("p (r c) -> p r c", c=3)
                    d3 = xt[i].rearrange("p (r c) -> p r c", c=3)
                    if bcast_mul:
                        s_b = sc[i].rearrange("p (r o) -> p r o", o=1).broadcast_to(
                            [P, r, 3]
                        )
                        nc.vector.tensor_mul(m3[:, :, :], d3[:, :, :], s_b)
                    else:
                        for k in range(3):
                            nc.vector.tensor_mul(m3[:, :, k], d3[:, :, k], sc[i][:, :])
                # G: out = m + c (in place over m), width split DVE/Pool
                i = t - SG
                if 0 <= i < n:
                    w = schedule[i]
                    w2 = int(w * add_split) // 96 * 96
                    if w2 > 0:
                        nc.vector.tensor_add(
                            sq[i][:, :w2], sq[i][:, :w2], ct[i][:, :w2]
                        )
                    if w2 < w:
                        nc.gpsimd.tensor_add(
                            sq[i][:, w2:], sq[i][:, w2:], ct[i][:, w2:]
                        )
                # H: output DMA
                i = t - SH
                if 0 <= i < n:
                    w, off = schedule[i], offs[i]
                    rings[out_ring].dma_start(orr[:, off : off + w], sq[i][:, :])

    nc.compile()
    return nc


_NC = None


def _get_nc():
    global _NC
    if _NC is None:
        _NC = _build()
    return _NC


def kernel(**inputs):
    x = np.asarray(inputs["x"], dtype=np.float32)
    center = np.asarray(inputs["center"], dtype=np.float32)
    assert x.shape == (B, 3) and center.shape == (B, 3)

    xs = x.reshape(N_CORES, B_CORE, 3)
    cs = center.reshape(N_CORES, B_CORE, 3)
    in_maps = [
        {"x": np.ascontiguousarray(xs[i]), "center": np.ascontiguousarray(cs[i])}
        for i in range(N_CORES)
    ]

    nc = _get_nc()
    res = run_bass_kernel_spmd(nc, in_maps, list(range(N_CORES)))
    out = np.concatenate([res.results[i]["out"] for i in range(N_CORES)], axis=0)
    return out.astype(np.float32, copy=False)


if __name__ == "__main__":
    nc = _get_nc()
    print("build ok")


# revision 8
# speedup vs baseline: 1.1728x; 1.1728x over previous
"""Trainium2 Bass kernel for CircleProjectionLayer (ball projection, r=1).

out = center + d * min(1, 1/||d||),  d = x - center,  shapes [8388608, 3] f32.

Sharding: pure data parallel — batch split 8 ways, one shard per NeuronCore.
Per-core layout: the [1048576, 3] shard is viewed flat as [128, 24576] so each
SBUF partition holds 8192 complete (x,y,z) rows contiguously; 13 chunks of
W=1920 floats per partition stream through SBUF (fewer chunks amortize
per-op fixed costs on DVE, the marginal engine).

The body is software-pipelined ("stage-skewed"): emission iteration t issues
stage S for chunk t - skew[S], so every engine's in-order queue only consumes
results produced >= 1 iteration earlier. This removes the head-of-line
blocking of naive per-chunk emission (engines ping-ponging DVE<->ACT within a
chunk), which measured ~55% above the pure-DMA floor (~113us/core for the
37.75 MB/core of HBM traffic at ~336 GB/s effective).

Stages per chunk i (W floats/partition, R = W/3 rows):
  A: dma x(i)->xt(i) [SP HWDGE ring], dma c(i)->ct(i) [ACT HWDGE ring]
  B: DVE  xt(i) = xt(i) - ct(i)                       (d, in place)
  C: ACT  sq(i) = Square(xt(i))
  D: DVE  ta(i) = sq0+sq1 ; tb(i) = max(ta,eps)+sq2   (scalar_tensor_tensor)
  E: ACT  Ln -> Relu -> Exp(-0.5*)  => sc(i)          (= min(1, rsqrt(ssq)),
     exact clamp at 1: relu(ln s)=0 for s<=1)
  F: DVE  sq(i) = d * broadcast(sc(i))                (one stride-0 mul over W)
  G: DVE  sq(i) += ct(i)   (out, in place; same iteration as F — same
     in-order engine, so no extra skew level needed)
  H: SP-ring dma out(i) <- sq(i)   (HWDGE)

GPSIMD runs NO ops at all: any Q7 tensor op measured a ~+20us intercept on
the total (regardless of width), and SWDGE descriptor generation on Q7 adds
more — so the final add lives on DVE (add_split=1.0) and the out-DMA on the
SP HWDGE ring. DVE busy/chunk ~7.5us vs the ~7.1us/chunk aggregate-HBM floor;
measured full-kernel 137.6us vs 174.2us unskewed baseline and ~113us
DMA-only floor.
"""

import sys

sys.path.insert(0, "/opt/trn_rl_repo")

from contextlib import ExitStack
import contextlib

import numpy as np

import concourse.bass as bass
import concourse.tile as tile
from concourse import bacc, mybir
from concourse.bass_utils import run_bass_kernel_spmd
from concourse.hw_specs import get_activation_tables

F32 = mybir.dt.float32
AF = mybir.ActivationFunctionType
ALU = mybir.AluOpType

B = 8388608
N_CORES = 8
B_CORE = B // N_CORES          # 1048576 rows per core
P = 128
FPP = B_CORE * 3 // P          # 24576 floats per partition

_EPS = 1e-30
_ACT_SET = "natural_log_exp_and_others"

_SCHEDULE = [1920] * 12 + [1536]


def _preload_act_table(nc):
    """Pre-place one LoadActFuncSet for the set containing Square/Ln/Relu/Exp
    so Bacc.insert_act_table_loads doesn't thrash between greedy choices."""
    tables = list(get_activation_tables(nc.m.arch).keys())
    set_id = tables.index(_ACT_SET)
    inst = mybir.InstLoadActFuncSet(
        name=nc.get_next_instruction_name(), act_func_set_id=set_id, ins=[], outs=[]
    )
    return nc.scalar.add_instruction(inst)


def _build(schedule=None, loop_reps=1,
           SB=2, SC=3, SD=4, SE=5, SF=6, SG=6, SH=7,
           bufs_xt=8, bufs_ct=8, bufs_sq=6, bufs_small=3,
           add_split=1.0, bcast_mul=True,
           x_ring="sp", c_ring="act", out_ring="sp"):
    """`loop_reps`: wrap the whole body in a hardware For_i loop (used only
    for benchmarking steady-state HW time via wall-clock deltas)."""
    if schedule is None:
        schedule = _SCHEDULE
    assert sum(schedule) == FPP and all(w % 3 == 0 for w in schedule)
    n = len(schedule)
    W = max(schedule)
    RW = W // 3

    nc = bacc.Bacc("TRN2", target_bir_lowering=False, debug=False)
    x = nc.dram_tensor("x", [B_CORE, 3], F32, kind="ExternalInput")
    c = nc.dram_tensor("center", [B_CORE, 3], F32, kind="ExternalInput")
    o = nc.dram_tensor("out", [B_CORE, 3], F32, kind="ExternalOutput")

    xr = x.ap().rearrange("(p f) c -> p (f c)", p=P)
    cr = c.ap().rearrange("(p f) c -> p (f c)", p=P)
    orr = o.ap().rearrange("(p f) c -> p (f c)", p=P)

    offs = [0]
    for w in schedule:
        offs.append(offs[-1] + w)

    with tile.TileContext(nc) as tc, ExitStack() as ctx:
        _preload_act_table(nc)
        xp = ctx.enter_context(tc.tile_pool(name="xp", bufs=bufs_xt))
        cp = ctx.enter_context(tc.tile_pool(name="cp", bufs=bufs_ct))
        sp = ctx.enter_context(tc.tile_pool(name="sp", bufs=bufs_sq))
        smp = ctx.enter_context(tc.tile_pool(name="smp", bufs=bufs_small))

        rings = {"sp": nc.sync, "act": nc.scalar, "pool": nc.gpsimd}

        loop_cm = tc.For_i(0, loop_reps, 1) if loop_reps > 1 else contextlib.nullcontext()
        with loop_cm:
            xt, ct, sq, ta, tb, sc = {}, {}, {}, {}, {}, {}
            for t in range(n + SH + 1):
                # A: input DMAs
                if t < n:
                    w, off = schedule[t], offs[t]
                    xt[t] = xp.tile([P, W], F32, name="xt", tag="xt")[:, :w]
                    rings[x_ring].dma_start(xt[t][:, :], xr[:, off : off + w])
                    ct[t] = cp.tile([P, W], F32, name="ct", tag="ct")[:, :w]
                    rings[c_ring].dma_start(ct[t][:, :], cr[:, off : off + w])
                # B: d = x - c (in place)
                i = t - SB
                if 0 <= i < n:
                    nc.vector.tensor_sub(xt[i][:, :], xt[i][:, :], ct[i][:, :])
                # C: squares
                i = t - SC
                if 0 <= i < n:
                    sq[i] = sp.tile([P, W], F32, name="sq", tag="sq")[:, : schedule[i]]
                    nc.scalar.activation(sq[i][:, :], xt[i][:, :], AF.Square)
                # D: row sums of squares
                i = t - SD
                if 0 <= i < n:
                    r = schedule[i] // 3
                    s3 = sq[i].rearrange("p (r c) -> p r c", c=3)
                    ta[i] = smp.tile([P, RW], F32, name="ta", tag="ta")[:, :r]
                    nc.vector.tensor_add(ta[i][:, :], s3[:, :, 0], s3[:, :, 1])
                    tb[i] = smp.tile([P, RW], F32, name="tb", tag="tb")[:, :r]
                    nc.vector.scalar_tensor_tensor(
                        tb[i][:, :], ta[i][:, :], _EPS, s3[:, :, 2], ALU.max, ALU.add
                    )
                # E: scale chain
                i = t - SE
                if 0 <= i < n:
                    r = schedule[i] // 3
                    nc.scalar.activation(ta[i][:, :], tb[i][:, :], AF.Ln)
                    nc.scalar.activation(tb[i][:, :], ta[i][:, :], AF.Relu)
                    sc[i] = smp.tile([P, RW], F32, name="sc", tag="sc")[:, :r]
                    nc.scalar.activation(sc[i][:, :], tb[i][:, :], AF.Exp, scale=-0.5)
                # F: m = d * s (one broadcast mul over the dead squares)
                i = t - SF
                if 0 <= i < n:
                    r = schedule[i] // 3
                    m3 = sq[i].rearrange("p (r c) -> p r c", c=3)
                    d3 = xt[i].rearrange("p (r c) -> p r c", c=3)
                    if bcast_mul:
                        s_b = sc[i].rearrange("p (r o) -> p r o", o=1).broadcast_to(
                            [P, r, 3]
                        )
                        nc.vector.tensor_mul(m3[:, :, :], d3[:, :, :], s_b)
                    else:
                        for k in range(3):
                            nc.vector.tensor_mul(m3[:, :, k], d3[:, :, k], sc[i][:, :])
                # G: out = m + c (in place over m), width split DVE/Pool
                i = t - SG
                if 0 <= i < n:
                    w = schedule[i]
                    w2 = int(w * add_split) // 96 * 96
                    if w2 > 0:
                        nc.vector.tensor_add(
                            sq[i][:, :w2], sq[i][:, :w2], ct[i][:, :w2]
                        )
                    if w2 < w:
                        nc.gpsimd.tensor_add(
                            sq[i][:, w2:], sq[i][:, w2:], ct[i][:, w2:]
                        )
                # H: output DMA
                i = t - SH
                if 0 <= i < n:
                    w, off = schedule[i], offs[i]
                    rings[out_ring].dma_start(orr[:, off : off + w], sq[i][:, :])

    nc.compile()
    return nc


_NC = None


def _get_nc():
    global _NC
    if _NC is None:
        _NC = _build()
    return _NC


def kernel(**inputs):
    x = np.asarray(inputs["x"], dtype=np.float32)
    center = np.asarray(inputs["center"], dtype=np.float32)
    assert x.shape == (B, 3) and center.shape == (B, 3)

    xs = x.reshape(N_CORES, B_CORE, 3)
    cs = center.reshape(N_CORES, B_CORE, 3)
    in_maps = [
        {"x": np.ascontiguousarray(xs[i]), "center": np.ascontiguousarray(cs[i])}
        for i in range(N_CORES)
    ]

    nc = _get_nc()
    res = run_bass_kernel_spmd(nc, in_maps, list(range(N_CORES)))
    out = np.concatenate([res.results[i]["out"] for i in range(N_CORES)], axis=0)
    return out.astype(np.float32, copy=False)


if __name__ == "__main__":
    nc = _get_nc()
    print("build ok")
